# revision 21
# baseline (speedup 1.0000x reference)
"""Bass/Tile kernel for nn_MultiHeadAttention (B=2, S=2048, D=1024, H=16).

Sharding: 8 cores = 2 (batch) x 4 (head-chunks of 4 heads).
Each core computes, for its batch b and its 4 heads:
  qpT/kpT = (x @ W{q,k} + b)^T   in [dout, token] layout (2 pairs of 128)
  vp      = v @ Wv + bv          in [token, dout] layout
  scoresT = kp @ qp^T            per head, [k, q] layout (row-tiled PE pairs)
  attnT   = exp(scoresT)         (softmax over q == free axis; no max-sub)
  Z[k]    = sum_q attnT[k, q]    (ACT accum_out per 1024-wide tile)
  hc      = sum_kb (vp[kb]/Z[kb])^T PV matmuls (col-tiled PE pairs),
            DVE-accumulated in SBUF f32r
  partial_p = hc_p^T @ Wo_p  -> per-pair [token, 1024] bf16 partial
              (host sums the 16 partials per batch)

Schedule: the kernel is ACT(exp)-bound in the attention phases and the PE
has a hardware duty-cycle throttle (~63% sustained), so PE work is spread:
the head runs all q/k projections as wide 1024-token chunks (through the
score PSUM slots, minimizing p-state ramp resets), the v-projection and
pair-0's O-projection trickle through the attention phases, and pair-1's
O-projection drains in a short tail on the freed PSUM banks.
"""

import sys

sys.path.insert(0, "/opt/trn_rl_repo")

from contextlib import ExitStack

import numpy as np
import ml_dtypes

import concourse.bass as bass
import concourse.mybir as mybir
import concourse.tile as tile
from concourse import bacc
from concourse.bass_utils import run_bass_kernel_spmd

BF16 = mybir.dt.bfloat16
F32 = mybir.dt.float32
F32R = mybir.dt.float32r
AF = mybir.ActivationFunctionType
ALU = mybir.AluOpType

D = 1024
NK = 8  # k-tiles over D
DOUT = 256  # per-core head dims (4 heads)
NPAIR = 2  # pairs of heads (128 dout each)
HD = 64


def build_kernel(S=2048):
    NKB = S // 128  # k-token blocks
    NQH = S // 1024  # exp blocks of 1024 along q
    NWC = S // 1024  # wide projection chunks
    NTT = S // 128  # token tiles
    assert S % 1024 == 0 and NQH == 2

    nc = bacc.Bacc("TRN2", target_bir_lowering=False, debug=False)

    qT = nc.dram_tensor("qT", [D, S], BF16, kind="ExternalInput")
    kT = nc.dram_tensor("kT", [D, S], BF16, kind="ExternalInput")
    vT = nc.dram_tensor("vT", [D, S], BF16, kind="ExternalInput")
    # weights/biases pre-laid-out host-side as [partition, ...] so the
    # resident loads are one contiguous DMA row per partition
    wq = nc.dram_tensor("wq", [128, NK, DOUT], BF16, kind="ExternalInput")
    wk = nc.dram_tensor("wk", [128, NK, DOUT], BF16, kind="ExternalInput")
    wv = nc.dram_tensor("wv", [128, NK, DOUT], BF16, kind="ExternalInput")
    wo = nc.dram_tensor("wo", [128, NPAIR, D], F32R, kind="ExternalInput")
    bq = nc.dram_tensor("bq", [128, NPAIR, 1], F32, kind="ExternalInput")
    bk = nc.dram_tensor("bk", [128, NPAIR, 1], F32, kind="ExternalInput")
    bv = nc.dram_tensor("bv", [DOUT], F32, kind="ExternalInput")
    out = nc.dram_tensor("out", [S, D], BF16, kind="ExternalOutput")

    # tiled DRAM views
    qTv = qT.ap().rearrange("(t p) s -> t p s", p=128)  # [8, 128, S]
    kTv = kT.ap().rearrange("(t p) s -> t p s", p=128)
    vTv = vT.ap().rearrange("(t p) s -> t p s", p=128)
    wqv = wq.ap()
    wkv = wk.ap()
    wvv = wv.ap()
    wov = wo.ap()
    bqv = bq.ap()
    bkv = bk.ap()
    outv = out.ap().rearrange("(t p) m -> t p m", p=128)  # [NTT, 128, D]

    bv_bcast_ap = bass.AP(tensor=bv.ap().tensor, offset=0, ap=[[0, 128], [1, DOUT]])

    with tile.TileContext(nc) as tc, ExitStack() as ctx:
        sb = ctx.enter_context(tc.tile_pool(name="sb", bufs=1))

        # ---- resident tiles ----
        wq_sb = sb.tile([128, NK, DOUT], BF16, tag="wq")
        wk_sb = sb.tile([128, NK, DOUT], BF16, tag="wk")
        wv_sb = sb.tile([128, NK, DOUT], BF16, tag="wv")
        wo_sb = sb.tile([128, NPAIR, D], F32R, tag="wo")
        bq_sb = sb.tile([128, NPAIR, 1], F32, tag="bq")
        bk_sb = sb.tile([128, NPAIR, 1], F32, tag="bk")
        bv_sb = sb.tile([128, DOUT], F32, tag="bv")
        vT_sb = sb.tile([128, NK, S], BF16, tag="vT")
        qpT_sb = sb.tile([128, NPAIR, S], BF16, tag="qpT")
        kpT_sb = sb.tile([128, NPAIR, S], BF16, tag="kpT")
        vp_sb = sb.tile([128, NTT, DOUT], BF16, tag="vp")
        # PV accumulator; f32r so DVE writes round for the f32r O-proj matmul
        hc_acc = sb.tile([128, NPAIR, S], F32R, tag="hc_acc")

        # q/k weights+biases first -- they gate the head
        nc.sync.dma_start(out=wq_sb[:], in_=wqv)
        nc.sync.dma_start(out=bq_sb[:], in_=bqv)
        nc.sync.dma_start(out=wk_sb[:], in_=wkv)
        nc.sync.dma_start(out=bk_sb[:], in_=bkv)
        nc.sync.dma_start(out=bv_sb[:], in_=bv_bcast_ap)

        # psum pool banks: sc0(2) + sc1(2) + pvt(2) + aux(2) = 8
        asb0 = ctx.enter_context(tc.tile_pool(name="qk_stream", bufs=1))
        psa = ctx.enter_context(tc.tile_pool(name="ps_all", bufs=1, space="PSUM"))
        asb = ctx.enter_context(tc.tile_pool(name="att_sb", bufs=1))
        osb = ctx.enter_context(tc.tile_pool(name="o_sb", bufs=1))

        WIDE_SLOTS = ["sc0", "sc1", "pvt"]

        # tiny dummy exp: pulls the ~2.7us ACT table load into the head
        dum = asb.tile([128, 1], F32, tag="dum", bufs=1, name="dum")
        dumz = asb.tile([128, 1], F32, tag="dumz", bufs=1, name="dumz")
        nc.scalar.activation(
            out=dum[:], in_=bv_sb[:, 0:1], func=AF.Exp, accum_out=dumz[:]
        )

        def emit_qkproj_wide(XTv, W_sb, b_sb, XPT, wc, slot_i, p):
            # one 1024-token chunk of a q/k projection for pair p, through
            # one of the wide (score/pv) psum slots
            tsl = slice(wc * 1024, wc * 1024 + 1024)
            ps_t = psa.tile(
                [128, 1024], F32, tag=WIDE_SLOTS[slot_i % 3], bufs=1,
                name="pjw",
            )
            for kk in range(NK):
                xt = asb0.tile([128, 1024], BF16, tag="xt", bufs=6, name="xt")
                nc.sync.dma_start(out=xt[:], in_=XTv[kk][:, tsl])
                for hf in range(2):
                    nc.tensor.matmul(
                        ps_t[:, hf * 512 : hf * 512 + 512],
                        lhsT=W_sb[:, kk, p * 128 : p * 128 + 128],
                        rhs=xt[:, hf * 512 : hf * 512 + 512],
                        start=(kk == 0),
                        stop=(kk == NK - 1),
                    )
            nc.vector.tensor_scalar_add(XPT[:, p, tsl], ps_t[:], b_sb[:, p, :])

        def emit_qkproj_narrow(XTv, W_sb, b_sb, XPT, tci, p):
            # 512-token chunk for one pair on the aux slots (attention-phase
            # trickle for pair 1)
            tsl = slice(tci * 512, tci * 512 + 512)
            ps_t = psa.tile([128, 512], F32, tag="aux", bufs=2, name=f"pj{p}")
            for kk in range(NK):
                xt = asb0.tile([128, 512], BF16, tag="xtn", bufs=12, name="xtn")
                nc.sync.dma_start(out=xt[:], in_=XTv[kk][:, tsl])
                nc.tensor.matmul(
                    ps_t[:],
                    lhsT=W_sb[:, kk, p * 128 : p * 128 + 128],
                    rhs=xt[:],
                    start=(kk == 0),
                    stop=(kk == NK - 1),
                )
            nc.vector.tensor_scalar_add(XPT[:, p, tsl], ps_t[:], b_sb[:, p, :])

        def emit_vproj(tt):
            psv = psa.tile([128, DOUT], F32, tag="aux", bufs=2, name="projv")
            for kk in range(NK):
                nc.tensor.matmul(
                    psv[:],
                    lhsT=vT_sb[:, kk, tt * 128 : tt * 128 + 128],
                    rhs=wv_sb[:, kk, :],
                    start=(kk == 0),
                    stop=(kk == NK - 1),
                )
            nc.vector.scalar_tensor_tensor(
                out=vp_sb[:, tt, :],
                in0=psv[:],
                scalar=1.0,
                in1=bv_sb[:],
                op0=ALU.mult,
                op1=ALU.add,
            )

        def emit_scores(p, kb):
            ksl = slice(kb * 128, kb * 128 + 128)
            scs = {}
            for qh in range(NQH):
                for h in range(2):
                    scs[(h, qh)] = psa.tile(
                        [128, 1024], F32, tag=f"sc{h}", bufs=1,
                        name=f"sc{h}{qh}",
                    )
                for qq in range(2):
                    for h in range(2):
                        hsl = slice(h * 64, h * 64 + 64)
                        qsl = slice(
                            qh * 1024 + qq * 512, qh * 1024 + qq * 512 + 512
                        )
                        nc.tensor.matmul(
                            scs[(h, qh)][:, qq * 512 : qq * 512 + 512],
                            lhsT=kpT_sb[hsl, p, ksl],
                            rhs=qpT_sb[hsl, p, qsl],
                            start=True,
                            stop=True,
                            tile_position=(h * 64, 0),
                        )
            return scs

        def emit_exps(scs):
            at_tiles, z_parts = {}, {}
            for h in range(2):
                for qh in range(NQH):
                    at = asb.tile(
                        [128, 1024], BF16, tag=f"at{h}_{qh}", bufs=3,
                        name=f"at{h}{qh}",
                    )
                    z = asb.tile(
                        [128, 1], F32, tag=f"z{h}_{qh}", bufs=3, name=f"z{h}{qh}"
                    )
                    nc.scalar.activation(
                        out=at[:], in_=scs[(h, qh)][:], func=AF.Exp,
                        accum_out=z[:],
                    )
                    at_tiles[(h, qh)] = at
                    z_parts[(h, qh)] = z
            return at_tiles, z_parts

        def emit_pv(p, kb, at_tiles, z_parts):
            vhss = {}
            for h in range(2):
                zs = asb.tile([128, 1], F32, tag=f"zs{h}", bufs=2, name="zs")
                nc.vector.tensor_add(zs[:], z_parts[(h, 0)][:], z_parts[(h, 1)][:])
                rz = asb.tile([128, 1], F32, tag=f"rz{h}", bufs=2, name="rz")
                nc.vector.reciprocal(rz[:], zs[:])
                vhs = asb.tile([128, HD], BF16, tag=f"vh{h}", bufs=2, name="vhs")
                nc.vector.tensor_scalar_mul(
                    vhs[:],
                    vp_sb[:, kb, p * 128 + h * 64 : p * 128 + h * 64 + 64],
                    rz[:],
                )
                vhss[h] = vhs
            for qh in range(NQH):
                pvt = psa.tile([128, 1024], F32, tag="pvt", bufs=1, name="pvt")
                for qq in range(2):
                    for h in range(2):
                        nc.tensor.matmul(
                            pvt[
                                h * 64 : h * 64 + 64,
                                qq * 512 : qq * 512 + 512,
                            ],
                            lhsT=vhss[h][:],
                            rhs=at_tiles[(h, qh)][
                                :, qq * 512 : qq * 512 + 512
                            ],
                            start=True,
                            stop=True,
                            tile_position=(0, h * 64),
                            skip_group_check=True,
                        )
                qsl = slice(qh * 1024, qh * 1024 + 1024)
                if kb == 0:
                    nc.vector.tensor_copy(hc_acc[:, p, qsl], pvt[:])
                else:
                    nc.vector.tensor_add(
                        hc_acc[:, p, qsl], hc_acc[:, p, qsl], pvt[:]
                    )

        def emit_oproj_wide(tt, slot):
            # [128, 1024] O-proj chunk (both pairs psum-accumulated) rotating
            # through the freed score/pv banks; copies alternate DVE/ACT
            # (tail only -- ACT is idle)
            ps_t = psa.tile([128, 1024], F32, tag=slot, bufs=1, name="ow")
            for dc in range(2):
                for p in range(NPAIR):
                    nc.tensor.matmul(
                        ps_t[:, dc * 512 : dc * 512 + 512],
                        lhsT=hc_acc[:, p, tt * 128 : tt * 128 + 128],
                        rhs=wo_sb[:, p, dc * 512 : dc * 512 + 512],
                        start=(p == 0),
                        stop=(p == NPAIR - 1),
                    )
            ost = osb.tile([128, 1024], BF16, tag="ostw", bufs=3, name="ostw")
            if tt % 2 == 0:
                nc.vector.tensor_copy(ost[:], ps_t[:])
            else:
                nc.scalar.copy(ost[:], ps_t[:])
            nc.sync.dma_start(out=outv[tt][:], in_=ost[:])

        # ---- head: pair-0 q/k projections as wide chunks; wide psum slots
        # rotate sc0/sc1/pvt. Pair-1 projections trickle in attention. ----
        si = 0
        for wc in range(NWC):
            emit_qkproj_wide(qTv, wq_sb, bq_sb, qpT_sb, wc, si, 0)
            si += 1
        for wc in range(NWC):
            emit_qkproj_wide(kTv, wk_sb, bk_sb, kpT_sb, wc, si, 0)
            si += 1

        # v inputs: first token-tiles ASAP (PV kb0 needs vp[0]), rest after
        nc.sync.dma_start(out=wv_sb[:], in_=wvv)
        for kk in range(NK):
            nc.sync.dma_start(out=vT_sb[:, kk, 0:256], in_=vTv[kk][:, 0:256])
        emit_vproj(0)
        emit_vproj(1)
        for kk in range(NK):
            nc.sync.dma_start(out=vT_sb[:, kk, 256:S], in_=vTv[kk][:, 256:S])
        nc.sync.dma_start(out=wo_sb[:], in_=wov)

        # ---- attention (ACT-bound); v-proj and pair-1 projections ride PE
        # slack during pair 0 ----
        for p in range(NPAIR):
            scs = emit_scores(p, 0)
            for kb in range(NKB):
                at_tiles, z_parts = emit_exps(scs)
                if kb + 1 < NKB:
                    scs = emit_scores(p, kb + 1)
                if p == 0:
                    if kb + 2 < NTT:
                        emit_vproj(kb + 2)
                    if kb < 4:
                        emit_qkproj_narrow(qTv, wq_sb, bq_sb, qpT_sb, kb, 1)
                    elif kb < 8:
                        emit_qkproj_narrow(kTv, wk_sb, bk_sb, kpT_sb, kb - 4, 1)
                emit_pv(p, kb, at_tiles, z_parts)

        # ---- tail: O-projection (both pairs) on the freed psum banks ----
        for tt in range(NTT):
            emit_oproj_wide(tt, WIDE_SLOTS[tt % 3])

    nc.compile()
    return nc


# ---------------- host-side shard / unshard ----------------

S = 2048
B = 2

_NC_CACHE = {}


def _get_nc():
    if "nc" not in _NC_CACHE:
        _NC_CACHE["nc"] = build_kernel(S=S)
    return _NC_CACHE["nc"]


def make_in_maps(q, k, v, Wq, bq, Wk, bk, Wv, bv, Wo, bo):
    bf = ml_dtypes.bfloat16
    maps = []
    for c in range(8):
        b = c // 4
        hc = c % 4
        cols = slice(256 * hc, 256 * hc + 256)
        maps.append({
            "qT": np.ascontiguousarray(q[b].astype(bf).T),
            "kT": np.ascontiguousarray(k[b].astype(bf).T),
            "vT": np.ascontiguousarray(v[b].astype(bf).T),
            "wq": np.ascontiguousarray(
                Wq[:, cols].reshape(NK, 128, DOUT).transpose(1, 0, 2).astype(bf)
            ),
            "wk": np.ascontiguousarray(
                Wk[:, cols].reshape(NK, 128, DOUT).transpose(1, 0, 2).astype(bf)
            ),
            "wv": np.ascontiguousarray(
                Wv[:, cols].reshape(NK, 128, DOUT).transpose(1, 0, 2).astype(bf)
            ),
            "wo": np.ascontiguousarray(
                Wo[cols, :].reshape(NPAIR, 128, D).transpose(1, 0, 2)
                .astype(np.float32)
            ),
            "bq": np.ascontiguousarray(
                bq[cols].reshape(NPAIR, 128, 1).transpose(1, 0, 2)
                .astype(np.float32)
            ),
            "bk": np.ascontiguousarray(
                bk[cols].reshape(NPAIR, 128, 1).transpose(1, 0, 2)
                .astype(np.float32)
            ),
            "bv": np.ascontiguousarray(bv[cols].astype(np.float32)),
        })
    return maps


def kernel(q, k, v, Wq, bq, Wk, bk, Wv, bv, Wo, bo):
    q = np.asarray(q, dtype=np.float32)
    k = np.asarray(k, dtype=np.float32)
    v = np.asarray(v, dtype=np.float32)
    Wq = np.asarray(Wq, dtype=np.float32)
    Wk = np.asarray(Wk, dtype=np.float32)
    Wv = np.asarray(Wv, dtype=np.float32)
    Wo = np.asarray(Wo, dtype=np.float32)
    bq = np.asarray(bq, dtype=np.float32)
    bk = np.asarray(bk, dtype=np.float32)
    bv = np.asarray(bv, dtype=np.float32)
    bo = np.asarray(bo, dtype=np.float32)

    nc = _get_nc()
    maps = make_in_maps(q, k, v, Wq, bq, Wk, bk, Wv, bv, Wo, bo)
    res = run_bass_kernel_spmd(nc, maps, core_ids=list(range(8)))

    outs = []
    for b in range(B):
        acc = np.zeros((S, D), dtype=np.float32)
        for hc in range(4):
            acc += res.results[b * 4 + hc]["out"].astype(np.float32)
        acc += bo[None, :]
        outs.append(acc)
    return np.stack(outs, axis=0)


# revision 22
# speedup vs baseline: 1.1414x; 1.1414x over previous
"""Fallback: batch-2 variant (measured 284.8us, rel err 0.0117).

All-bf16 operands, original software-pipelined structure, row-tiled scores,
col-tiled PV, combined O-projection with fp32 partial output per core.
"""

import sys

sys.path.insert(0, "/opt/trn_rl_repo")

from contextlib import ExitStack

import numpy as np
import ml_dtypes

import concourse.bass as bass
import concourse.mybir as mybir
import concourse.tile as tile
from concourse import bacc
from concourse.bass_utils import run_bass_kernel_spmd

BF16 = mybir.dt.bfloat16
F32 = mybir.dt.float32
F32R = mybir.dt.float32r
AF = mybir.ActivationFunctionType
ALU = mybir.AluOpType

D = 1024
NK = 8
DOUT = 256
NPAIR = 2
HD = 64


def build_kernel(S=2048, use_tile_position="scores_too"):
    NKB = S // 128
    NQH = S // 1024
    NTC = S // 512
    NTT = S // 128
    assert S % 1024 == 0

    nc = bacc.Bacc("TRN2", target_bir_lowering=False, debug=False)

    qT = nc.dram_tensor("qT", [D, S], BF16, kind="ExternalInput")
    kT = nc.dram_tensor("kT", [D, S], BF16, kind="ExternalInput")
    vT = nc.dram_tensor("vT", [D, S], BF16, kind="ExternalInput")
    wq = nc.dram_tensor("wq", [D, DOUT], BF16, kind="ExternalInput")
    wk = nc.dram_tensor("wk", [D, DOUT], BF16, kind="ExternalInput")
    wv = nc.dram_tensor("wv", [D, DOUT], BF16, kind="ExternalInput")
    wo = nc.dram_tensor("wo", [DOUT, D], F32R, kind="ExternalInput")
    bq = nc.dram_tensor("bq", [NPAIR, 128, 1], F32, kind="ExternalInput")
    bk = nc.dram_tensor("bk", [NPAIR, 128, 1], F32, kind="ExternalInput")
    bv = nc.dram_tensor("bv", [DOUT], F32, kind="ExternalInput")
    out = nc.dram_tensor("out", [S, D], F32, kind="ExternalOutput")

    qTv = qT.ap().rearrange("(t p) s -> t p s", p=128)
    kTv = kT.ap().rearrange("(t p) s -> t p s", p=128)
    vTv = vT.ap().rearrange("(t p) s -> t p s", p=128)
    wqv = wq.ap().rearrange("(t p) m -> p t m", p=128)
    wkv = wk.ap().rearrange("(t p) m -> p t m", p=128)
    wvv = wv.ap().rearrange("(t p) m -> p t m", p=128)
    wov = wo.ap().rearrange("(t p) m -> p t m", p=128)
    bqv = bq.ap().rearrange("a p o -> p a o")
    bkv = bk.ap().rearrange("a p o -> p a o")
    outv = out.ap().rearrange("(t p) m -> t p m", p=128)

    bv_bcast_ap = bass.AP(tensor=bv.ap().tensor, offset=0, ap=[[0, 128], [1, DOUT]])

    with tile.TileContext(nc) as tc, ExitStack() as ctx:
        sb = ctx.enter_context(tc.tile_pool(name="sb", bufs=1))

        wq_sb = sb.tile([128, NK, DOUT], BF16, tag="wq")
        wk_sb = sb.tile([128, NK, DOUT], BF16, tag="wk")
        wv_sb = sb.tile([128, NK, DOUT], BF16, tag="wv")
        wo_sb = sb.tile([128, NPAIR, D], F32R, tag="wo")
        nc.sync.dma_start(out=wq_sb[:], in_=wqv)
        nc.sync.dma_start(out=wk_sb[:], in_=wkv)
        bq_sb = sb.tile([128, NPAIR, 1], F32, tag="bq")
        bk_sb = sb.tile([128, NPAIR, 1], F32, tag="bk")
        bv_sb = sb.tile([128, DOUT], F32, tag="bv")
        nc.sync.dma_start(out=bq_sb[:], in_=bqv)
        nc.sync.dma_start(out=bk_sb[:], in_=bkv)
        nc.sync.dma_start(out=bv_sb[:], in_=bv_bcast_ap)

        vT_sb = sb.tile([128, NK, S], BF16, tag="vT")

        qpT_sb = sb.tile([128, NPAIR, S], BF16, tag="qpT")
        kpT_sb = sb.tile([128, NPAIR, S], BF16, tag="kpT")
        vp_sb = sb.tile([128, NTT, DOUT], BF16, tag="vp")

        hc_acc = sb.tile([128, NPAIR, S], F32R, tag="hc_acc")

        asb0 = ctx.enter_context(tc.tile_pool(name="qk_stream", bufs=1))
        psa = ctx.enter_context(tc.tile_pool(name="ps_all", bufs=1, space="PSUM"))
        asb = ctx.enter_context(tc.tile_pool(name="att_sb", bufs=1))

        def emit_qkproj(XTv, W_sb, b_sb, XPT, tci):
            tsl = slice(tci * 512, tci * 512 + 512)
            pss = []
            for p in range(NPAIR):
                ps_t = psa.tile([128, 512], F32, tag="aux", bufs=2, name=f"pj{p}")
                pss.append(ps_t)
            for kk in range(NK):
                xt = asb0.tile([128, 512], BF16, tag="xt", bufs=12, name="xt")
                nc.sync.dma_start(out=xt[:], in_=XTv[kk][:, tsl])
                for p in range(NPAIR):
                    nc.tensor.matmul(
                        pss[p][:],
                        lhsT=W_sb[:, kk, p * 128 : p * 128 + 128],
                        rhs=xt[:],
                        start=(kk == 0),
                        stop=(kk == NK - 1),
                    )
            for p in range(NPAIR):
                nc.vector.tensor_scalar_add(XPT[:, p, tsl], pss[p][:], b_sb[:, p, :])

        def emit_vproj(tt):
            psv = psa.tile([128, DOUT], F32, tag="aux", bufs=2, name="projv")
            for kk in range(NK):
                nc.tensor.matmul(
                    psv[:],
                    lhsT=vT_sb[:, kk, tt * 128 : tt * 128 + 128],
                    rhs=wv_sb[:, kk, :],
                    start=(kk == 0),
                    stop=(kk == NK - 1),
                )
            nc.vector.scalar_tensor_tensor(
                out=vp_sb[:, tt, :],
                in0=psv[:],
                scalar=1.0,
                in1=bv_sb[:],
                op0=ALU.mult,
                op1=ALU.add,
            )

        def emit_scores(p, kb):
            ksl = slice(kb * 128, kb * 128 + 128)
            scs = {}
            for qh in range(NQH):
                for h in range(2):
                    scs[(h, qh)] = psa.tile(
                        [128, 1024], F32, tag=f"sc{h}", bufs=1,
                        name=f"sc{h}{qh}",
                    )
                for qq in range(2):
                    for h in range(2):
                        hsl = slice(h * 64, h * 64 + 64)
                        qsl = slice(
                            qh * 1024 + qq * 512, qh * 1024 + qq * 512 + 512
                        )
                        nc.tensor.matmul(
                            scs[(h, qh)][:, qq * 512 : qq * 512 + 512],
                            lhsT=kpT_sb[hsl, p, ksl],
                            rhs=qpT_sb[hsl, p, qsl],
                            start=True,
                            stop=True,
                            tile_position=(h * 64, 0)
                            if use_tile_position == "scores_too"
                            else None,
                        )
            return scs

        def emit_exps(scs):
            at_tiles, z_parts = {}, {}
            for h in range(2):
                for qh in range(NQH):
                    at = asb.tile(
                        [128, 1024], BF16, tag=f"at{h}_{qh}", bufs=3,
                        name=f"at{h}{qh}",
                    )
                    z = asb.tile(
                        [128, 1], F32, tag=f"z{h}_{qh}", bufs=3, name=f"z{h}{qh}"
                    )
                    nc.scalar.activation(
                        out=at[:], in_=scs[(h, qh)][:], func=AF.Exp,
                        accum_out=z[:],
                    )
                    at_tiles[(h, qh)] = at
                    z_parts[(h, qh)] = z
            return at_tiles, z_parts

        def emit_pv(p, kb, at_tiles, z_parts):
            vhss = {}
            for h in range(2):
                if NQH > 1:
                    zs = asb.tile([128, 1], F32, tag=f"zs{h}", bufs=2, name="zs")
                    nc.vector.tensor_add(
                        zs[:], z_parts[(h, 0)][:], z_parts[(h, 1)][:]
                    )
                    for qh in range(2, NQH):
                        nc.vector.tensor_add(zs[:], zs[:], z_parts[(h, qh)][:])
                else:
                    zs = z_parts[(h, 0)]
                rz = asb.tile([128, 1], F32, tag=f"rz{h}", bufs=2, name="rz")
                nc.vector.reciprocal(rz[:], zs[:])
                vhs = asb.tile([128, HD], BF16, tag=f"vh{h}", bufs=2, name="vhs")
                nc.vector.tensor_scalar_mul(
                    vhs[:],
                    vp_sb[:, kb, p * 128 + h * 64 : p * 128 + h * 64 + 64],
                    rz[:],
                )
                vhss[h] = vhs
            for qh in range(NQH):
                pvt = psa.tile([128, 1024], F32, tag="pvt", bufs=1, name="pvt")
                for qq in range(2):
                    for h in range(2):
                        nc.tensor.matmul(
                            pvt[
                                h * 64 : h * 64 + 64,
                                qq * 512 : qq * 512 + 512,
                            ],
                            lhsT=vhss[h][:],
                            rhs=at_tiles[(h, qh)][
                                :, qq * 512 : qq * 512 + 512
                            ],
                            start=True,
                            stop=True,
                            tile_position=(0, h * 64),
                            skip_group_check=True,
                        )
                qsl = slice(qh * 1024, qh * 1024 + 1024)
                if kb == 0:
                    nc.vector.tensor_copy(hc_acc[:, p, qsl], pvt[:])
                else:
                    nc.vector.tensor_add(
                        hc_acc[:, p, qsl], hc_acc[:, p, qsl], pvt[:]
                    )


        osb = ctx.enter_context(tc.tile_pool(name="o_sb", bufs=1))

        for tci in range(NTC):
            emit_qkproj(qTv, wq_sb, bq_sb, qpT_sb, tci)
        for tci in range(NTC):
            emit_qkproj(kTv, wk_sb, bk_sb, kpT_sb, tci)

        nc.sync.dma_start(out=wv_sb[:], in_=wvv)
        for kk in range(NK):
            nc.sync.dma_start(out=vT_sb[:, kk, :], in_=vTv[kk])
        nc.sync.dma_start(out=wo_sb[:], in_=wov)

        for p in range(NPAIR):
            scs = emit_scores(p, 0)
            if p == 0:
                emit_vproj(0)
            for kb in range(NKB):
                at_tiles, z_parts = emit_exps(scs)
                if kb + 1 < NKB:
                    scs = emit_scores(p, kb + 1)
                if p == 0 and kb + 1 < NKB:
                    emit_vproj(kb + 1)
                emit_pv(p, kb, at_tiles, z_parts)

        hcT = hc_acc
        for tt in range(NTT):
            for dc in range(2):
                ps_t = psa.tile([128, 512], F32, tag="aux", bufs=2, name=f"o{dc}")
                for p in range(NPAIR):
                    nc.tensor.matmul(
                        ps_t[:],
                        lhsT=hcT[:, p, tt * 128 : tt * 128 + 128],
                        rhs=wo_sb[:, p, dc * 512 : dc * 512 + 512],
                        start=(p == 0),
                        stop=(p == NPAIR - 1),
                    )
                ost = osb.tile(
                    [128, 512], F32, tag=f"ost{dc}", bufs=2, name=f"ost{dc}"
                )
                if (tt + dc) % 2 == 0:
                    nc.vector.tensor_copy(ost[:], ps_t[:])
                else:
                    nc.scalar.copy(ost[:], ps_t[:])
                nc.sync.dma_start(
                    out=outv[tt][:, dc * 512 : dc * 512 + 512], in_=ost[:]
                )

    nc.compile()
    return nc


S = 2048
B = 2

_NC_CACHE = {}


def _get_nc():
    if "nc" not in _NC_CACHE:
        _NC_CACHE["nc"] = build_kernel(S=S, use_tile_position="scores_too")
    return _NC_CACHE["nc"]


def make_in_maps(q, k, v, Wq, bq, Wk, bk, Wv, bv, Wo, bo):
    bf = ml_dtypes.bfloat16
    maps = []
    for c in range(8):
        b = c // 4
        hc = c % 4
        cols = slice(256 * hc, 256 * hc + 256)
        maps.append({
            "qT": np.ascontiguousarray(q[b].astype(bf).T),
            "kT": np.ascontiguousarray(k[b].astype(bf).T),
            "vT": np.ascontiguousarray(v[b].astype(bf).T),
            "wq": np.ascontiguousarray(Wq[:, cols].astype(bf)),
            "wk": np.ascontiguousarray(Wk[:, cols].astype(bf)),
            "wv": np.ascontiguousarray(Wv[:, cols].astype(bf)),
            "wo": np.ascontiguousarray(Wo[cols, :].astype(np.float32)),
            "bq": np.ascontiguousarray(
                bq[cols].reshape(NPAIR, 128, 1).astype(np.float32)
            ),
            "bk": np.ascontiguousarray(
                bk[cols].reshape(NPAIR, 128, 1).astype(np.float32)
            ),
            "bv": np.ascontiguousarray(bv[cols].astype(np.float32)),
        })
    return maps


def kernel(q, k, v, Wq, bq, Wk, bk, Wv, bv, Wo, bo):
    q = np.asarray(q, dtype=np.float32)
    k = np.asarray(k, dtype=np.float32)
    v = np.asarray(v, dtype=np.float32)
    Wq = np.asarray(Wq, dtype=np.float32)
    Wk = np.asarray(Wk, dtype=np.float32)
    Wv = np.asarray(Wv, dtype=np.float32)
    Wo = np.asarray(Wo, dtype=np.float32)
    bq = np.asarray(bq, dtype=np.float32)
    bk = np.asarray(bk, dtype=np.float32)
    bv = np.asarray(bv, dtype=np.float32)
    bo = np.asarray(bo, dtype=np.float32)

    nc = _get_nc()
    maps = make_in_maps(q, k, v, Wq, bq, Wk, bk, Wv, bv, Wo, bo)
    res = run_bass_kernel_spmd(nc, maps, core_ids=list(range(8)))

    outs = []
    for b in range(B):
        acc = np.zeros((S, D), dtype=np.float32)
        for hc in range(4):
            acc += res.results[b * 4 + hc]["out"]
        acc += bo[None, :]
        outs.append(acc)
    return np.stack(outs, axis=0)


# revision 24
# speedup vs baseline: 1.1709x; 1.0259x over previous
"""Fallback: batch-2 variant (measured 284.8us, rel err 0.0117).

All-bf16 operands, original software-pipelined structure, row-tiled scores,
col-tiled PV, combined O-projection with fp32 partial output per core.
"""

import sys

sys.path.insert(0, "/opt/trn_rl_repo")

from contextlib import ExitStack

import numpy as np
import ml_dtypes

import concourse.bass as bass
import concourse.mybir as mybir
import concourse.tile as tile
from concourse import bacc
from concourse.bass_utils import run_bass_kernel_spmd

BF16 = mybir.dt.bfloat16
F32 = mybir.dt.float32
F32R = mybir.dt.float32r
AF = mybir.ActivationFunctionType
ALU = mybir.AluOpType

D = 1024
NK = 8
DOUT = 256
NPAIR = 2
HD = 64


def build_kernel(S=2048, use_tile_position="scores_too"):
    NKB = S // 128
    NQH = S // 1024
    NTC = S // 512
    NTT = S // 128
    assert S % 1024 == 0

    nc = bacc.Bacc("TRN2", target_bir_lowering=False, debug=False)

    qT = nc.dram_tensor("qT", [D, S], BF16, kind="ExternalInput")
    kT = nc.dram_tensor("kT", [D, S], BF16, kind="ExternalInput")
    vT = nc.dram_tensor("vT", [D, S], BF16, kind="ExternalInput")
    wq = nc.dram_tensor("wq", [D, DOUT], BF16, kind="ExternalInput")
    wk = nc.dram_tensor("wk", [D, DOUT], BF16, kind="ExternalInput")
    wv = nc.dram_tensor("wv", [D, DOUT], BF16, kind="ExternalInput")
    wo = nc.dram_tensor("wo", [DOUT, D], F32R, kind="ExternalInput")
    bq = nc.dram_tensor("bq", [NPAIR, 128, 1], F32, kind="ExternalInput")
    bk = nc.dram_tensor("bk", [NPAIR, 128, 1], F32, kind="ExternalInput")
    bv = nc.dram_tensor("bv", [DOUT], F32, kind="ExternalInput")
    out = nc.dram_tensor("out", [S, D], F32, kind="ExternalOutput")

    qTv = qT.ap().rearrange("(t p) s -> t p s", p=128)
    kTv = kT.ap().rearrange("(t p) s -> t p s", p=128)
    vTv = vT.ap().rearrange("(t p) s -> t p s", p=128)
    wqv = wq.ap().rearrange("(t p) m -> p t m", p=128)
    wkv = wk.ap().rearrange("(t p) m -> p t m", p=128)
    wvv = wv.ap().rearrange("(t p) m -> p t m", p=128)
    wov = wo.ap().rearrange("(t p) m -> p t m", p=128)
    bqv = bq.ap().rearrange("a p o -> p a o")
    bkv = bk.ap().rearrange("a p o -> p a o")
    outv = out.ap().rearrange("(t p) m -> t p m", p=128)

    bv_bcast_ap = bass.AP(tensor=bv.ap().tensor, offset=0, ap=[[0, 128], [1, DOUT]])

    with tile.TileContext(nc) as tc, ExitStack() as ctx:
        sb = ctx.enter_context(tc.tile_pool(name="sb", bufs=1))

        wq_sb = sb.tile([128, NK, DOUT], BF16, tag="wq")
        wk_sb = sb.tile([128, NK, DOUT], BF16, tag="wk")
        wv_sb = sb.tile([128, NK, DOUT], BF16, tag="wv")
        wo_sb = sb.tile([128, NPAIR, D], F32R, tag="wo")
        nc.sync.dma_start(out=wq_sb[:], in_=wqv)
        nc.sync.dma_start(out=wk_sb[:], in_=wkv)
        bq_sb = sb.tile([128, NPAIR, 1], F32, tag="bq")
        bk_sb = sb.tile([128, NPAIR, 1], F32, tag="bk")
        bv_sb = sb.tile([128, DOUT], F32, tag="bv")
        nc.sync.dma_start(out=bq_sb[:], in_=bqv)
        nc.sync.dma_start(out=bk_sb[:], in_=bkv)
        nc.sync.dma_start(out=bv_sb[:], in_=bv_bcast_ap)

        vT_sb = sb.tile([128, NK, S], BF16, tag="vT")

        qpT_sb = sb.tile([128, NPAIR, S], BF16, tag="qpT")
        kpT_sb = sb.tile([128, NPAIR, S], BF16, tag="kpT")
        vp_sb = sb.tile([128, NTT, DOUT], BF16, tag="vp")

        hc_acc = sb.tile([128, NPAIR, S], F32R, tag="hc_acc")

        asb0 = ctx.enter_context(tc.tile_pool(name="qk_stream", bufs=1))
        psa = ctx.enter_context(tc.tile_pool(name="ps_all", bufs=1, space="PSUM"))
        asb = ctx.enter_context(tc.tile_pool(name="att_sb", bufs=1))

        def emit_qkproj(XTv, W_sb, b_sb, XPT, tci, pairs=(0, 1)):
            tsl = slice(tci * 512, tci * 512 + 512)
            pss = {}
            for p in pairs:
                ps_t = psa.tile([128, 512], F32, tag="aux", bufs=2, name=f"pj{p}")
                pss[p] = ps_t
            for kk in range(NK):
                xt = asb0.tile([128, 512], BF16, tag="xt", bufs=12, name="xt")
                nc.sync.dma_start(out=xt[:], in_=XTv[kk][:, tsl])
                for p in pairs:
                    nc.tensor.matmul(
                        pss[p][:],
                        lhsT=W_sb[:, kk, p * 128 : p * 128 + 128],
                        rhs=xt[:],
                        start=(kk == 0),
                        stop=(kk == NK - 1),
                    )
            for p in pairs:
                nc.vector.tensor_scalar_add(XPT[:, p, tsl], pss[p][:], b_sb[:, p, :])

        def emit_vproj(tt):
            psv = psa.tile([128, DOUT], F32, tag="aux", bufs=2, name="projv")
            for kk in range(NK):
                nc.tensor.matmul(
                    psv[:],
                    lhsT=vT_sb[:, kk, tt * 128 : tt * 128 + 128],
                    rhs=wv_sb[:, kk, :],
                    start=(kk == 0),
                    stop=(kk == NK - 1),
                )
            nc.vector.scalar_tensor_tensor(
                out=vp_sb[:, tt, :],
                in0=psv[:],
                scalar=1.0,
                in1=bv_sb[:],
                op0=ALU.mult,
                op1=ALU.add,
            )

        def emit_scores(p, kb):
            ksl = slice(kb * 128, kb * 128 + 128)
            scs = {}
            for qh in range(NQH):
                for h in range(2):
                    scs[(h, qh)] = psa.tile(
                        [128, 1024], F32, tag=f"sc{h}", bufs=1,
                        name=f"sc{h}{qh}",
                    )
                for qq in range(2):
                    for h in range(2):
                        hsl = slice(h * 64, h * 64 + 64)
                        qsl = slice(
                            qh * 1024 + qq * 512, qh * 1024 + qq * 512 + 512
                        )
                        nc.tensor.matmul(
                            scs[(h, qh)][:, qq * 512 : qq * 512 + 512],
                            lhsT=kpT_sb[hsl, p, ksl],
                            rhs=qpT_sb[hsl, p, qsl],
                            start=True,
                            stop=True,
                            tile_position=(h * 64, 0)
                            if use_tile_position == "scores_too"
                            else None,
                        )
            return scs

        def emit_exps(scs):
            at_tiles, z_parts = {}, {}
            for h in range(2):
                for qh in range(NQH):
                    at = asb.tile(
                        [128, 1024], BF16, tag=f"at{h}_{qh}", bufs=3,
                        name=f"at{h}{qh}",
                    )
                    z = asb.tile(
                        [128, 1], F32, tag=f"z{h}_{qh}", bufs=3, name=f"z{h}{qh}"
                    )
                    nc.scalar.activation(
                        out=at[:], in_=scs[(h, qh)][:], func=AF.Exp,
                        accum_out=z[:],
                    )
                    at_tiles[(h, qh)] = at
                    z_parts[(h, qh)] = z
            return at_tiles, z_parts

        def emit_pv(p, kb, at_tiles, z_parts):
            vhss = {}
            for h in range(2):
                if NQH > 1:
                    zs = asb.tile([128, 1], F32, tag=f"zs{h}", bufs=2, name="zs")
                    nc.vector.tensor_add(
                        zs[:], z_parts[(h, 0)][:], z_parts[(h, 1)][:]
                    )
                    for qh in range(2, NQH):
                        nc.vector.tensor_add(zs[:], zs[:], z_parts[(h, qh)][:])
                else:
                    zs = z_parts[(h, 0)]
                rz = asb.tile([128, 1], F32, tag=f"rz{h}", bufs=2, name="rz")
                nc.vector.reciprocal(rz[:], zs[:])
                vhs = asb.tile([128, HD], BF16, tag=f"vh{h}", bufs=2, name="vhs")
                nc.vector.tensor_scalar_mul(
                    vhs[:],
                    vp_sb[:, kb, p * 128 + h * 64 : p * 128 + h * 64 + 64],
                    rz[:],
                )
                vhss[h] = vhs
            for qh in range(NQH):
                pvt = psa.tile([128, 1024], F32, tag="pvt", bufs=1, name="pvt")
                for qq in range(2):
                    for h in range(2):
                        nc.tensor.matmul(
                            pvt[
                                h * 64 : h * 64 + 64,
                                qq * 512 : qq * 512 + 512,
                            ],
                            lhsT=vhss[h][:],
                            rhs=at_tiles[(h, qh)][
                                :, qq * 512 : qq * 512 + 512
                            ],
                            start=True,
                            stop=True,
                            tile_position=(0, h * 64),
                            skip_group_check=True,
                        )
                qsl = slice(qh * 1024, qh * 1024 + 1024)
                if kb == 0:
                    nc.vector.tensor_copy(hc_acc[:, p, qsl], pvt[:])
                else:
                    nc.vector.tensor_add(
                        hc_acc[:, p, qsl], hc_acc[:, p, qsl], pvt[:]
                    )


        osb = ctx.enter_context(tc.tile_pool(name="o_sb", bufs=1))

        # head: pair-0 projections only -- attention (ACT) starts ~25us in.
        # Pair-1 projections are deferred to kb8-15 of pair-0's attention,
        # after the PE's DVFS ramp, so the early (slow-clock) kbs carry only
        # the tile-paired scores/PV plus the JIT v-projection.
        for tci in range(NTC):
            emit_qkproj(qTv, wq_sb, bq_sb, qpT_sb, tci, pairs=(0,))
        for tci in range(NTC):
            emit_qkproj(kTv, wk_sb, bk_sb, kpT_sb, tci, pairs=(0,))

        nc.sync.dma_start(out=wv_sb[:], in_=wvv)
        for kk in range(NK):
            nc.sync.dma_start(out=vT_sb[:, kk, :], in_=vTv[kk])
        nc.sync.dma_start(out=wo_sb[:], in_=wov)

        for p in range(NPAIR):
            scs = emit_scores(p, 0)
            if p == 0:
                emit_vproj(0)
            for kb in range(NKB):
                at_tiles, z_parts = emit_exps(scs)
                if kb + 1 < NKB:
                    scs = emit_scores(p, kb + 1)
                if p == 0 and kb + 1 < NKB:
                    emit_vproj(kb + 1)
                if p == 0 and 8 <= kb < 12:
                    emit_qkproj(qTv, wq_sb, bq_sb, qpT_sb, kb - 8, pairs=(1,))
                elif p == 0 and 12 <= kb:
                    emit_qkproj(kTv, wk_sb, bk_sb, kpT_sb, kb - 12, pairs=(1,))
                emit_pv(p, kb, at_tiles, z_parts)

        hcT = hc_acc
        for tt in range(NTT):
            for dc in range(2):
                ps_t = psa.tile([128, 512], F32, tag="aux", bufs=2, name=f"o{dc}")
                for p in range(NPAIR):
                    nc.tensor.matmul(
                        ps_t[:],
                        lhsT=hcT[:, p, tt * 128 : tt * 128 + 128],
                        rhs=wo_sb[:, p, dc * 512 : dc * 512 + 512],
                        start=(p == 0),
                        stop=(p == NPAIR - 1),
                    )
                ost = osb.tile(
                    [128, 512], F32, tag=f"ost{dc}", bufs=2, name=f"ost{dc}"
                )
                if (tt + dc) % 2 == 0:
                    nc.vector.tensor_copy(ost[:], ps_t[:])
                else:
                    nc.scalar.copy(ost[:], ps_t[:])
                nc.sync.dma_start(
                    out=outv[tt][:, dc * 512 : dc * 512 + 512], in_=ost[:]
                )

    nc.compile()
    return nc


S = 2048
B = 2

_NC_CACHE = {}


def _get_nc():
    if "nc" not in _NC_CACHE:
        _NC_CACHE["nc"] = build_kernel(S=S, use_tile_position="scores_too")
    return _NC_CACHE["nc"]


def make_in_maps(q, k, v, Wq, bq, Wk, bk, Wv, bv, Wo, bo):
    bf = ml_dtypes.bfloat16
    maps = []
    for c in range(8):
        b = c // 4
        hc = c % 4
        cols = slice(256 * hc, 256 * hc + 256)
        maps.append({
            "qT": np.ascontiguousarray(q[b].astype(bf).T),
            "kT": np.ascontiguousarray(k[b].astype(bf).T),
            "vT": np.ascontiguousarray(v[b].astype(bf).T),
            "wq": np.ascontiguousarray(Wq[:, cols].astype(bf)),
            "wk": np.ascontiguousarray(Wk[:, cols].astype(bf)),
            "wv": np.ascontiguousarray(Wv[:, cols].astype(bf)),
            "wo": np.ascontiguousarray(Wo[cols, :].astype(np.float32)),
            "bq": np.ascontiguousarray(
                bq[cols].reshape(NPAIR, 128, 1).astype(np.float32)
            ),
            "bk": np.ascontiguousarray(
                bk[cols].reshape(NPAIR, 128, 1).astype(np.float32)
            ),
            "bv": np.ascontiguousarray(bv[cols].astype(np.float32)),
        })
    return maps


def kernel(q, k, v, Wq, bq, Wk, bk, Wv, bv, Wo, bo):
    q = np.asarray(q, dtype=np.float32)
    k = np.asarray(k, dtype=np.float32)
    v = np.asarray(v, dtype=np.float32)
    Wq = np.asarray(Wq, dtype=np.float32)
    Wk = np.asarray(Wk, dtype=np.float32)
    Wv = np.asarray(Wv, dtype=np.float32)
    Wo = np.asarray(Wo, dtype=np.float32)
    bq = np.asarray(bq, dtype=np.float32)
    bk = np.asarray(bk, dtype=np.float32)
    bv = np.asarray(bv, dtype=np.float32)
    bo = np.asarray(bo, dtype=np.float32)

    nc = _get_nc()
    maps = make_in_maps(q, k, v, Wq, bq, Wk, bk, Wv, bv, Wo, bo)
    res = run_bass_kernel_spmd(nc, maps, core_ids=list(range(8)))

    outs = []
    for b in range(B):
        acc = np.zeros((S, D), dtype=np.float32)
        for hc in range(4):
            acc += res.results[b * 4 + hc]["out"]
        acc += bo[None, :]
        outs.append(acc)
    return np.stack(outs, axis=0)
